# revision 7
# baseline (speedup 1.0000x reference)
"""Trainium2 Bass kernel for nn_AD_Embedding (dense_mlp).

Math (per scalar x, shared tiny weights):
  y0 = leaky_relu(x * W1)                       # [30]
  z  = (Wl + 0.1 I) @ y0                        # [30]
  p  = softmax(0.5 * z)                         # [30]
  out = W2 @ p                                  # [100]

Host-side folding: leaky_relu(w*x) is linear in the basis (x, relu(x)) with
per-output coefficients depending on sign(w), so stages 1+2 collapse into
  zh = A*x + Bv*relu(x),  A = G@a, Bv = G@b, G = 0.5*(Wl + 0.1 I)
and p = softmax(zh) with the temperature folded in.

Device layout (per core, 61440 rows = 2048 batch x 30 features):
  - rows are processed in 30 "macro" chunks of 2048 rows = 4 blocks x 512
  - z computed bins-on-partitions as [128, 512] (4 row-blocks of 32
    partitions each, 30 used) by 2 accumulating K=4 block-diagonal matmuls
  - e = exp(z) on ScalarE -> bf16 [128, 512]
  - final matmul flipped: stationary = e column-slice [128,128], moving =
    block-diagonal W2 (+ per-block ones column for the softmax sum):
    out u [128, 404] lands rows-on-partitions, ready for contiguous DMA
  - normalize: reciprocal of the 4 sum columns, 16x tensor_scalar into SBUF
  - one 4D-strided DMA per macro writes [128, 1600] -> DRAM
"""

import numpy as np
import ml_dtypes

import concourse.bass as bass
import concourse.tile as tile
from concourse import bacc, mybir
from concourse.bass_utils import run_bass_kernel_spmd

B, F, BINS, EMB = 16384, 30, 30, 100
T = 0.5
N_CORES = 8
ROWS = B * F // N_CORES          # 61440 rows per core
NCHUNK = ROWS // 512             # 120 chunks of 512 rows
NMACRO = NCHUNK // 4             # 30 macros of 2048 rows
BF16 = mybir.dt.bfloat16
F32 = mybir.dt.float32
npbf16 = ml_dtypes.bfloat16

_CACHE = {}


def _build():
    nc = bacc.Bacc("TRN2", target_bir_lowering=False, debug=False,
                   num_devices=N_CORES)
    x_ext = nc.dram_tensor("x", [120, 512], BF16, kind="ExternalInput").ap()
    m2_ext = nc.dram_tensor("m2", [8, 128], BF16, kind="ExternalInput").ap()
    w2a_ext = nc.dram_tensor("w2a", [128, 404], BF16, kind="ExternalInput").ap()
    out_ext = nc.dram_tensor("out", [ROWS, EMB], F32, kind="ExternalOutput").ap()

    # out flat row index = 2048*m + 512*j + 128*t + p
    out5 = out_ext.rearrange("(m j t p) e -> m j t p e", m=NMACRO, j=4, t=4, p=128)

    AF = mybir.ActivationFunctionType

    with tile.TileContext(nc) as tc:
        with (
            tc.tile_pool(name="consts", bufs=1) as consts,
            tc.tile_pool(name="xpool", bufs=1) as xpool,
            tc.tile_pool(name="zp", bufs=2, space="PSUM") as zpool,
            tc.tile_pool(name="up", bufs=4, space="PSUM") as upool,
            tc.tile_pool(name="ep", bufs=3) as epool,
            tc.tile_pool(name="rsp", bufs=8) as rspool,
            tc.tile_pool(name="op", bufs=3) as opool,
        ):
            m2 = consts.tile([8, 128], BF16, tag="m2")
            nc.sync.dma_start(m2[:], m2_ext[:])
            w2a = consts.tile([128, 404], BF16, tag="w2a")
            nc.sync.dma_start(w2a[:], w2a_ext[:])

            xall = xpool.tile([120, 512], BF16, tag="xall")
            nc.sync.dma_start(xall[:], x_ext[:])
            rall = xpool.tile([120, 512], BF16, tag="rall")
            nc.scalar.activation(rall[:], xall[:], AF.Relu)

            for m in range(NMACRO):
                xr = epool.tile([8, 512], BF16, tag="xr")
                nc.sync.dma_start(xr[0:4, :], xall[4 * m:4 * m + 4, :])
                nc.sync.dma_start(xr[4:8, :], rall[4 * m:4 * m + 4, :])
                zp = zpool.tile([128, 512], F32, tag="zp")
                nc.tensor.matmul(zp[:], lhsT=m2[:], rhs=xr[:],
                                 start=True, stop=True)
                e = epool.tile([128, 512], BF16, tag="e")
                nc.scalar.activation(e[:], zp[:], AF.Exp)

                outT = opool.tile([128, 1600], F32, tag="outT")
                for t in range(4):
                    u = upool.tile([128, 404], F32, tag="u")
                    nc.tensor.matmul(u[:], lhsT=e[:, 128 * t:128 * t + 128],
                                     rhs=w2a[:], start=True, stop=True)
                    rs = rspool.tile([128, 4], F32, tag="rs")
                    for j in range(4):
                        nc.vector.reciprocal(rs[:, j:j + 1],
                                             u[:, 101 * j + 100:101 * j + 101])
                    for j in range(4):
                        o0 = 400 * j + 100 * t  # (j,t)-major so DMA dims merge
                        nc.vector.tensor_scalar_mul(
                            outT[:, o0:o0 + 100],
                            u[:, 101 * j:101 * j + 100],
                            rs[:, j:j + 1])

                out_dst = out5[m].rearrange("j t p e -> p (j t) e")
                out_src = outT[:].rearrange("p (jt e) -> p jt e", e=100)
                nc.sync.dma_start(out_dst, out_src)

    nc.compile()
    return nc


def _host_prep(x, W1, Wl, W2):
    W1f = W1[:, 0].astype(np.float64)
    a = np.where(W1f >= 0, 0.01 * W1f, W1f)
    b = np.where(W1f >= 0, 0.99 * W1f, -0.99 * W1f)
    G = T * (Wl.astype(np.float64) + 0.1 * np.eye(BINS))
    A = (G @ a).astype(np.float32)
    Bv = (G @ b).astype(np.float32)

    # M2 [8, 128]: rows 0-3 = x-coefs per block, rows 4-7 = relu-coefs;
    # block j occupies stationary columns 32j..32j+30
    m2 = np.zeros((8, 128), np.float32)
    for j in range(4):
        m2[j, 32 * j:32 * j + 30] = A
        m2[4 + j, 32 * j:32 * j + 30] = Bv

    # W2aug [128, 404]: rows 32j..32j+30 hold W2^T for block j in columns
    # 101j..101j+100, plus a ones column at 101j+100 for the softmax sum.
    w2a = np.zeros((128, 404), np.float32)
    for j in range(4):
        w2a[32 * j:32 * j + 30, 101 * j:101 * j + 100] = W2.T
        w2a[32 * j:32 * j + 30, 101 * j + 100] = 1.0

    return (m2.astype(npbf16), w2a.astype(npbf16))


def kernel(x, W1, Wl, W2):
    if "nc" not in _CACHE:
        _CACHE["nc"] = _build()
    nc = _CACHE["nc"]

    m2, w2a = _host_prep(x, W1, Wl, W2)
    xflat = np.ascontiguousarray(x.reshape(B * F)).astype(npbf16)  # r = 30b+f
    in_maps = []
    for c in range(N_CORES):
        xs = xflat[c * ROWS:(c + 1) * ROWS].reshape(120, 512)
        in_maps.append({"x": xs, "m2": m2, "w2a": w2a})

    res = run_bass_kernel_spmd(nc, in_maps, core_ids=list(range(N_CORES)))
    parts = [res.results[c]["out"].reshape(B // N_CORES, F * EMB)
             for c in range(N_CORES)]
    return np.concatenate(parts, axis=0)
